# revision 5
# baseline (speedup 1.0000x reference)
"""Sparse (half-causal) multi-head attention on 8 Trainium2 NeuronCores, v2.

Problem: x[2,2048,1024] -> QKV proj (16 heads, dk=dv=64) -> scores with
half-causal mask (rows <1024 attend cols <1024 dense; rows >=1024 causal)
-> softmax -> out proj.

Sharding: 8 cores = 2 batches x 4 head-groups (4 heads each).  Each core
computes its batch's full QKV for its 4 heads (column-sharded W), attention
for those heads, and a partial output projection (row-sharded Wo).  Host
sums the 4 partials per batch.

v2 changes vs v1 (178.9us):
 - all matmul inputs bf16 (same PE rate as f32r in the cost model, half the
   DMA traffic and SBUF footprint; emulated rel-err ~6e-3 vs the 2e-2 gate)
 - K bias dropped entirely: softmax is invariant to per-query constants,
   and (q+bq)@(k+bk) = (q+bq)@k + const(q)
 - causal staircase trimmed at 128-column granularity (scores, exp and PV
   all skip the fully-masked region; only the one diagonal 128-block needs
   a [128,128] mask multiply)
 - scores/PV software-pipelined one kc deep (also across head blocks) so
   the in-order PE stream never stalls on exp; QKV chunk-2/3 and out-proj
   matmuls woven in as filler where exp is the local bottleneck
 - norm multiplies deferred ~one head so the in-order DVE stream never
   blocks on the denominator-broadcast DMA roundtrip
 - finer ramp: split xt/wq DMAs so the first Q matmul starts ~2us in
"""

import sys

if "/opt/trn_rl_repo" not in sys.path:
    sys.path.insert(0, "/opt/trn_rl_repo")

import ml_dtypes
import numpy as np

import concourse.bass as bass  # noqa: F401 (import registers engines)
import concourse.mybir as mybir
import concourse.tile as tile
from concourse import bacc
from concourse.bass_utils import run_bass_kernel_spmd

f32 = mybir.dt.float32
bf16 = mybir.dt.bfloat16
AF = mybir.ActivationFunctionType
OP = mybir.AluOpType

D = 1024  # d_model
N = 2048  # n_ctx
HG = 256  # head-group width per core (4 heads x 64)

HEADS = [(0, 0), (0, 1), (1, 0), (1, 1)]  # (hp, par)
SEG_LAST = {0: (7, 7), 1: (11, 15)}  # last kc accumulating into each q-seg


def build_nc():
    nc = bacc.Bacc("TRN2", target_bir_lowering=False, debug=False)

    xt = nc.declare_dram_parameter("xt", [D, N], bf16, isOutput=False)
    wq = nc.declare_dram_parameter("wq", [D, HG], bf16, isOutput=False)
    wk = nc.declare_dram_parameter("wk", [D, HG], bf16, isOutput=False)
    wv = nc.declare_dram_parameter("wv", [D, HG], bf16, isOutput=False)
    bqd = nc.declare_dram_parameter("bq", [HG], f32, isOutput=False)
    bvd = nc.declare_dram_parameter("bv", [HG], bf16, isOutput=False)
    wo = nc.declare_dram_parameter("wo", [HG, D], bf16, isOutput=False)
    trid = nc.declare_dram_parameter("tri", [128, 128], bf16, isOutput=False)
    onesd = nc.declare_dram_parameter("ones", [512], bf16, isOutput=False)
    y = nc.declare_dram_parameter("y", [N, D], bf16, isOutput=True)
    y2 = nc.declare_dram_parameter("y2", [1024, D], bf16, isOutput=True)

    dscr = nc.dram_tensor("dscr", [2, 2, 2, 1024], bf16)  # (hp, par, half, q)

    xt_r = xt[:].rearrange("(c p) n -> p c n", p=128)
    wq_r = wq[:].rearrange("(c p) m -> p c m", p=128)
    wk_r = wk[:].rearrange("(c p) m -> p c m", p=128)
    wv_r = wv[:].rearrange("(c p) m -> p c m", p=128)
    wo_r = wo[:].rearrange("(c p) n -> p c n", p=128)

    with tile.TileContext(nc) as tc:
        with (
            tc.tile_pool(name="persist", bufs=1) as P1,
            tc.tile_pool(name="xtp", bufs=6) as XTP,
            tc.tile_pool(name="ppool", bufs=6) as PP,
            tc.tile_pool(name="rp", bufs=3) as RP,
            tc.tile_pool(name="rbp", bufs=2) as RBP,
            tc.tile_pool(name="atp", bufs=4) as ATP,
            tc.tile_pool(name="yp", bufs=4) as YP,
            tc.tile_pool(name="ps_a", bufs=2, space="PSUM") as PSA,
            tc.tile_pool(name="ps_pv", bufs=2, space="PSUM") as PSPV,
            tc.tile_pool(name="ps_b", bufs=2, space="PSUM") as PSB,
        ):
            # ---------- persistent tiles ----------
            wq_sb = P1.tile([128, 8, HG], bf16, tag="wq")
            wk_sb = P1.tile([128, 8, HG], bf16, tag="wk")
            wv_sb = P1.tile([128, 8, HG], bf16, tag="wv")
            wo_sb = P1.tile([128, 2, D], bf16, tag="wo")
            bq_sb = P1.tile([128, 2], f32, tag="bq")
            bv_sb = P1.tile([1, HG], bf16, tag="bv")
            ones_sb = P1.tile([1, 512], bf16, tag="ones")
            tri_sb = P1.tile([128, 128], bf16, tag="tri")

            qT = P1.tile([128, 2, N], bf16, tag="qT")
            kT = P1.tile([128, 2, N], bf16, tag="kT")
            v1 = P1.tile([128, 16, 4, 65], bf16, tag="v1")
            att = P1.tile([128, 2, N], bf16, tag="att")

            # ---------- QKV emitters ----------
            xt_tiles = {}

            def emit_load(n4, split=False):
                ns = slice(512 * n4, 512 * n4 + 512)
                a = XTP.tile([128, 4, 512], bf16, tag="xt", name=f"xta{n4}")
                b_ = XTP.tile([128, 4, 512], bf16, tag="xt", name=f"xtb{n4}")
                for h in range(2):
                    cs = slice(2 * h, 2 * h + 2)
                    nc.sync.dma_start(a[:, cs, :], xt_r[:, cs, ns])
                # first chunk: b-half streams on the gpsimd queue in parallel
                eng = nc.gpsimd if split else nc.sync
                for h in range(2):
                    cs = slice(2 * h, 2 * h + 2)
                    eng.dma_start(b_[:, cs, :], xt_r[:, 4 + 2 * h : 6 + 2 * h, ns])
                xt_tiles[n4] = (a, b_)

            def emit_q(n4, m):
                ns = slice(512 * n4, 512 * n4 + 512)
                a, b_ = xt_tiles[n4]
                msl = slice(128 * m, 128 * m + 128)
                ps = PSB.tile([128, 512], f32, tag="b", name=f"qps{n4}{m}")
                for c in range(8):
                    nc.tensor.matmul(
                        ps[:],
                        wq_sb[:, c, msl],
                        (a if c < 4 else b_)[:, c % 4, :],
                        start=(c == 0),
                        stop=(c == 7),
                    )
                with nc.allow_low_precision(reason="bf16 qT"):
                    nc.vector.tensor_scalar_add(qT[:, m, ns], ps[:], bq_sb[:, m : m + 1])

            def emit_k(n4, m, act_copy=False):
                ns = slice(512 * n4, 512 * n4 + 512)
                a, b_ = xt_tiles[n4]
                msl = slice(128 * m, 128 * m + 128)
                ps = PSB.tile([128, 512], f32, tag="b", name=f"kps{n4}{m}")
                for c in range(8):
                    nc.tensor.matmul(
                        ps[:],
                        wk_sb[:, c, msl],
                        (a if c < 4 else b_)[:, c % 4, :],
                        start=(c == 0),
                        stop=(c == 7),
                    )
                with nc.allow_low_precision(reason="bf16 kT"):
                    if act_copy:
                        nc.scalar.copy(out=kT[:, m, ns], in_=ps[:])
                    else:
                        nc.vector.tensor_copy(out=kT[:, m, ns], in_=ps[:])

            def emit_v(s, act_copy=False):
                n4 = s // 4
                a, b_ = xt_tiles[n4]
                so = 128 * (s - 4 * n4)
                ps = PSB.tile([128, 256], f32, tag="b", name=f"vps{s}")
                for c in range(8):
                    nc.tensor.matmul(
                        ps[:],
                        (a if c < 4 else b_)[:, c % 4, so : so + 128],
                        wv_sb[:, c, :],
                        start=(c == 0),
                        stop=False,
                    )
                nc.tensor.matmul(
                    ps[:], ones_sb[:, :128], bv_sb[:], start=False, stop=True
                )
                with nc.allow_low_precision(reason="bf16 v1"):
                    src = ps[:].rearrange("p (h d) -> p h d", h=4)
                    if act_copy:
                        nc.scalar.copy(out=v1[:, s, :, 0:64], in_=src)
                    else:
                        nc.vector.tensor_copy(out=v1[:, s, :, 0:64], in_=src)

            # ---------- attention ----------
            pv_tiles = {}

            def emit_scores(hp, par, half, kc):
                q0 = 1024 * half
                base = 64 * par
                o = 0 if (half == 0 or kc < 8) else 128 * (kc - 8)
                s_t = PSA.tile(
                    [128, 1024], f32, tag="s", name=f"s{hp}{par}{half}{kc}"
                )
                mm = [(o, 512), (512, 1024)] if o < 512 else [(o, 1024)]
                for lo, hi in mm:
                    nc.tensor.matmul(
                        s_t[:, lo:hi],
                        kT[base : base + 64, hp, 128 * kc : 128 * kc + 128],
                        qT[base : base + 64, hp, q0 + lo : q0 + hi],
                        start=True,
                        stop=True,
                    )
                p_t = PP.tile([128, 1024], bf16, tag="p", name=f"p{hp}{par}{half}{kc}")
                with nc.allow_low_precision(reason="bf16 probs"):
                    nc.scalar.activation(
                        p_t[:, o:1024], s_t[:, o:1024], AF.Exp, scale=0.125
                    )
                if half == 1 and kc >= 8:
                    nc.vector.tensor_tensor(
                        p_t[:, o : o + 128], p_t[:, o : o + 128], tri_sb[:], OP.mult
                    )
                return p_t

            def emit_pv(hp, par, half, kc, p_t):
                if kc == 0:
                    pv_tiles[(hp, par, half)] = [
                        PSPV.tile(
                            [65, 512], f32, tag="pv", name=f"pv{hp}{par}{half}{i}"
                        )
                        for i in range(2)
                    ]
                pv = pv_tiles[(hp, par, half)]
                o = 0 if (half == 0 or kc < 8) else 128 * (kc - 8)
                masked = half == 1 and kc >= 8
                # unmasked region first so PV doesn't wait on the DVE mask
                pieces = []
                lo0 = o + 128 if masked else o
                if lo0 < 512:
                    pieces.append((lo0, 512, 0))
                if max(lo0, 512) < 1024:
                    pieces.append((max(lo0, 512), 1024, 1))
                if masked:
                    pieces.append((o, o + 128, 0 if o < 512 else 1))
                last = SEG_LAST[half]
                for i, (lo, hi, seg) in enumerate(pieces):
                    is_last_of_seg = all(p[2] != seg for p in pieces[i + 1 :])
                    nc.tensor.matmul(
                        pv[seg][0:65, lo - 512 * seg : hi - 512 * seg],
                        v1[:, kc, 2 * hp + par, :],
                        p_t[:, lo:hi],
                        start=(kc == 0),
                        stop=(kc == last[seg] and is_last_of_seg),
                        skip_group_check=(half == 1),
                    )

            # one-deep pipeline: PV (+post hook) for the previous scores is
            # emitted right after the next scores, including across blocks
            pend = []  # [(hp, par, half, kc, p_t, hook)]

            def flush_pend():
                while pend:
                    hp, par, half, kc, p_t, hook = pend.pop(0)
                    emit_pv(hp, par, half, kc, p_t)
                    if hook:
                        hook()

            def attn_step(hp, par, half, kc, hook=None):
                p_t = emit_scores(hp, par, half, kc)
                flush_pend()
                pend.append((hp, par, half, kc, p_t, hook))

            def attn_block(hp, par, half, kcs, filler=(), every=3, hooks=None):
                """Pops filler every `every` kc, plus every kc in the narrow
                exp-bound diagonal tail (kc>=12) where PE work shrinks."""
                hooks = hooks or {}
                filler = list(filler)
                for i, kc in enumerate(kcs):
                    attn_step(hp, par, half, kc, hooks.get(kc))
                    if filler and (
                        i == 1
                        or (i > 1 and (i - 1) % every == 0)
                        or (half == 1 and kc >= 12)
                    ):
                        filler.pop(0)()
                while filler:
                    filler.pop(0)()

            # ---------- softmax normalization ----------
            deferred = []  # delayed norm multiplies (DVE)

            def flush_norms():
                while deferred:
                    deferred.pop(0)()

            def emit_norm(hp, par, half, seg, fast=False, stage_act=False):
                q0 = 1024 * half + 512 * seg
                base = 64 * par
                pv = pv_tiles[(hp, par, half)][seg]
                sl = slice(512 * seg, 512 * seg + 512)
                stage = ATP.tile(
                    [65, 512], bf16, tag="st", name=f"st{hp}{par}{half}{seg}"
                )
                with nc.allow_low_precision(reason="bf16 stage"):
                    # ACT staging at block boundaries: frees the PSUM pv bank
                    # without queueing behind filler copies on DVE
                    if stage_act:
                        nc.scalar.copy(out=stage[:], in_=pv[:, :])
                    else:
                        nc.vector.tensor_copy(out=stage[:], in_=pv[:, :])
                r = RP.tile([1, 512], bf16, tag="r", name=f"r{hp}{par}{half}{seg}")
                with nc.allow_low_precision(reason="bf16 denom"):
                    nc.vector.reciprocal(r[:], stage[64:65, :])
                if fast:
                    rb = PSB.tile(
                        [64, 512], f32, tag="b", name=f"rbf{hp}{par}{half}{seg}"
                    )
                    nc.tensor.matmul(rb[:], ones_sb[:, :64], r[:], start=True, stop=True)
                else:
                    nc.sync.dma_start(dscr[hp, par, half, sl], r[:])
                    rb = RBP.tile(
                        [64, 512], bf16, tag="rb", name=f"rb{hp}{par}{half}{seg}"
                    )
                    nc.sync.dma_start(
                        rb[:], dscr[hp, par, half, sl].partition_broadcast(64)
                    )

                def mult():
                    with nc.allow_low_precision(reason="bf16 att"):
                        nc.vector.tensor_tensor(
                            att[base : base + 64, hp, q0 : q0 + 512],
                            stage[0:64, :],
                            rb[:],
                            OP.mult,
                        )

                if fast:
                    mult()
                else:
                    deferred.append(mult)

            qn_pend = {}

            def quarter_stage(hp, par, kc):
                """Last-head fast path: the 128-col q-block ending at this kc
                (chunk 8+kc-8) is fully accumulated once PV(kc) lands, so it
                can normalize while later diagonal kc's are still running.
                Three-stage pipeline (stage/recip -> rb matmul -> multiply)
                so each PE step only consumes DVE results from >=1 kc ago."""
                qn = kc - 8
                seg, off = qn // 4, 128 * (qn % 4)
                pv = pv_tiles[(hp, par, 1)][seg]
                stq = ATP.tile([65, 128], bf16, tag="stq", name=f"stq{hp}{par}{qn}")
                with nc.allow_low_precision(reason="bf16 stage"):
                    nc.vector.tensor_copy(out=stq[:], in_=pv[0:65, off : off + 128])
                rq = RP.tile([1, 128], bf16, tag="rq", name=f"rq{hp}{par}{qn}")
                with nc.allow_low_precision(reason="bf16 denom"):
                    nc.vector.reciprocal(rq[:], stq[64:65, :])
                qn_pend[kc] = [stq, rq, None]

            def quarter_rb(hp, par, kc):
                qn = kc - 8
                stq, rq, _ = qn_pend[kc]
                rb = PSB.tile([64, 128], f32, tag="b", name=f"rbq{hp}{par}{qn}")
                nc.tensor.matmul(rb[:], ones_sb[:, :64], rq[:], start=True, stop=True)
                qn_pend[kc][2] = rb

            def quarter_norm_finish(hp, par, kc):
                qn = kc - 8
                base = 64 * par
                stq, rq, rb = qn_pend.pop(kc)
                q0 = 1024 + 128 * qn
                with nc.allow_low_precision(reason="bf16 att"):
                    nc.vector.tensor_tensor(
                        att[base : base + 64, hp, q0 : q0 + 128],
                        stq[0:64, :],
                        rb[:],
                        OP.mult,
                    )

            # ---------- output projection ----------
            def emit_outproj(s, hps, split_copies=False, seg_dma=False, pools=None):
                yt = YP.tile([128, D], bf16, tag="y", name=f"yt{s}h{hps[0]}{len(hps)}")
                tgt = (
                    y2[128 * (s - 8) : 128 * (s - 8) + 128, :]
                    if hps == (1,)
                    else y[128 * s : 128 * s + 128, :]
                )
                for nseg in range(2):
                    pool = (pools or (PSB, PSB))[nseg]
                    ptag = "b" if pool is PSB else ("pv" if pool is PSPV else "s")
                    ps = pool.tile(
                        [128, 512], f32, tag=ptag,
                        name=f"yps{s}{nseg}h{hps[0]}{len(hps)}",
                    )
                    for i, hp in enumerate(hps):
                        nc.tensor.matmul(
                            ps[:],
                            att[:, hp, 128 * s : 128 * s + 128],
                            wo_sb[:, hp, 512 * nseg : 512 * nseg + 512],
                            start=(i == 0),
                            stop=(i == len(hps) - 1),
                        )
                    sl = slice(512 * nseg, 512 * nseg + 512)
                    with nc.allow_low_precision(reason="bf16 y"):
                        if split_copies and nseg == 0:
                            nc.scalar.copy(out=yt[:, sl], in_=ps[:])
                        else:
                            nc.vector.tensor_copy(out=yt[:, sl], in_=ps[:])
                # dual-queue issue in the tail: the ~650ns/DMA sequencer cost
                # would otherwise serialize the final drain on one queue
                # (gpsimd's SWDGE path is slower end-to-end: keep it off the
                # last chunks)
                eng = nc.gpsimd if (seg_dma and s < 14) else nc.sync
                eng.dma_start(tgt, yt[:])

            # ============================================================
            # emission schedule
            # ============================================================
            # --- ramp: stream x/weights, QKV chunks 0-1 ---
            # weights go out on the scalar DGE queue so they stream in
            # parallel with the xt chunks on the sync queue
            nc.gpsimd.dma_start(ones_sb[:], onesd[None, :])
            nc.scalar.dma_start(wq_sb[:, 0:4, :], wq_r[:, 0:4, :])
            emit_load(0, split=True)
            nc.scalar.dma_start(wq_sb[:, 4:8, :], wq_r[:, 4:8, :])
            nc.scalar.dma_start(wk_sb[:, 0:4, :], wk_r[:, 0:4, :])
            nc.scalar.dma_start(wk_sb[:, 4:8, :], wk_r[:, 4:8, :])
            nc.scalar.dma_start(wv_sb[:], wv_r[:])
            nc.gpsimd.dma_start(bq_sb[:], bqd[:].rearrange("(m p) -> p m", p=128))
            nc.gpsimd.dma_start(bv_sb[:], bvd[None, :])
            nc.scalar.dma_start(tri_sb[:], trid[:])
            # warmup matmuls: climb the PE p-state while DMAs stream (the
            # first ~3us of continuous PE execution runs below peak clock)
            for w in range(8):
                junk = PSB.tile([128, 512], f32, tag="b", name=f"warm{w}")
                nc.tensor.matmul(
                    junk[:], ones_sb[0:1, :128], ones_sb[:], start=True, stop=True
                )
            emit_q(0, 0)
            emit_q(0, 1)
            emit_k(0, 0, act_copy=True)
            emit_load(1)
            emit_k(0, 1, act_copy=True)
            nc.sync.dma_start(v1[:, :, :, 64:65], onesd[0:64].partition_broadcast(128))
            for s in range(4):
                emit_v(s, act_copy=(s % 2 == 0))
            emit_q(1, 0)
            emit_q(1, 1)
            emit_k(1, 0, act_copy=True)
            emit_k(1, 1, act_copy=True)
            for s in range(4, 8):
                emit_v(s, act_copy=(s % 2 == 0))
            emit_load(2)
            emit_load(3)
            nc.scalar.dma_start(wo_sb[:], wo_r[:])

            # --- QKV chunk 2/3 filler (Q first: half-1 scores need qT;
            #     K/V chunks 2-3 are only needed for the kc>=8 diagonal) ---
            qkv23 = (
                [lambda n4=n4, m=m: emit_q(n4, m) for n4 in (2, 3) for m in (0, 1)]
                + [lambda n4=n4, m=m: emit_k(n4, m) for n4 in (2, 3) for m in (0, 1)]
                + [lambda s=s: emit_v(s) for s in range(8, 12)]
            )
            v_late = [lambda s=s: emit_v(s) for s in range(12, 16)]

            # --- attention q-half 0 (dense) ---
            for hp, par in HEADS:

                def h0norms(hp=hp, par=par):
                    emit_norm(hp, par, 0, 0)
                    emit_norm(hp, par, 0, 1)

                flush_norms()
                attn_block(
                    hp, par, 0, range(8), qkv23[:3], every=2, hooks={7: h0norms}
                )
                qkv23 = qkv23[3:]
            flush_pend()  # last head's PV + its norms
            while qkv23:  # drain leftovers (PE-bound, ACT idle)
                qkv23.pop(0)()
            flush_norms()

            # --- attention q-half 1 (dense kc0-7 + causal diag kc8-15) ---
            op07 = [lambda s=s: emit_outproj(s, (0, 1)) for s in range(8)]
            oph0 = [lambda s=s: emit_outproj(s, (0,)) for s in range(8, 16)]

            def h1_head(hp, par, filler, fast=False, extra_hooks=None, every=3):
                def norm0():
                    emit_norm(hp, par, 1, 0, fast=fast)

                def norm1():
                    emit_norm(hp, par, 1, 1, fast=fast)

                # flush at kc0's PV (after the previous head's stage copies,
                # so they aren't queued behind a blocking norm multiply)
                hooks = {0: flush_norms, 11: norm0, 15: norm1}
                if extra_hooks:
                    for k, fn in extra_hooks.items():
                        prev_fn = hooks.get(k)

                        def both(prev_fn=prev_fn, fn=fn):
                            if prev_fn:
                                prev_fn()
                            fn()

                        hooks[k] = both
                attn_block(hp, par, 1, range(16), filler, every=every, hooks=hooks)

            h1_head(0, 0, v_late + op07[:3], every=2)
            h1_head(0, 1, op07[3:])
            h1_head(1, 0, oph0[:7], extra_hooks={6: flush_norms}, every=2)

            # last head: per-quarter norms ride the causal staircase — the
            # hp1 out-projection pipelines against the remaining diagonal
            def q_finish(kc):
                s = kc
                quarter_norm_finish(1, 1, kc)
                pools = (PSB, PSB) if s < 12 else (PSPV, PSB)
                emit_outproj(
                    s, (1,), split_copies=True, seg_dma=(s >= 12), pools=pools
                )

            def q_hook(kc):
                def fn():
                    if kc == 8:
                        flush_norms()  # (1,0) seg0/seg1 multiplies
                    if kc >= 10:
                        q_finish(kc - 2)
                    if kc >= 9:
                        quarter_rb(1, 1, kc - 1)
                    quarter_stage(1, 1, kc)

                return fn

            lhooks = {kc: q_hook(kc) for kc in range(8, 16)}
            attn_block(1, 1, 1, range(16), oph0[7:], hooks={0: flush_norms, **lhooks})
            flush_pend()  # PV(15) + quarter stage(15)
            quarter_rb(1, 1, 15)
            q_finish(14)
            q_finish(15)
            flush_norms()

    nc.compile()
    return nc


_NC = None


def _get_nc():
    global _NC
    if _NC is None:
        _NC = build_nc()
    return _NC


def make_in_maps(x, Wq, bq, Wk, bk, Wv, bv, Wo):
    _get_nc()
    bf = ml_dtypes.bfloat16
    x = np.asarray(x, np.float32)
    kk = np.arange(128)[:, None]
    qp = np.arange(128)[None, :]
    tri = (kk <= qp).astype(bf)
    ones = np.ones(512, bf)
    in_maps = []
    for core in range(8):
        b, g = core // 4, core % 4
        sl = slice(HG * g, HG * (g + 1))
        in_maps.append(
            {
                "xt": np.ascontiguousarray(x[b].T.astype(bf)),
                "wq": np.ascontiguousarray(np.asarray(Wq, np.float32)[:, sl].astype(bf)),
                "wk": np.ascontiguousarray(np.asarray(Wk, np.float32)[:, sl].astype(bf)),
                "wv": np.ascontiguousarray(np.asarray(Wv, np.float32)[:, sl].astype(bf)),
                "bq": np.ascontiguousarray(np.asarray(bq, np.float32)[sl]),
                "bv": np.ascontiguousarray(np.asarray(bv, np.float32)[sl].astype(bf)),
                "wo": np.ascontiguousarray(np.asarray(Wo, np.float32)[sl, :].astype(bf)),
                "tri": tri,
                "ones": ones,
            }
        )
    return in_maps


def kernel(x, Wq, bq, Wk, bk, Wv, bv, Wo, _trace=False, _trace_kwargs=None):
    nc = _get_nc()
    in_maps = make_in_maps(x, Wq, bq, Wk, bk, Wv, bv, Wo)
    res = run_bass_kernel_spmd(
        nc, in_maps, list(range(8)), trace=_trace, **(_trace_kwargs or {})
    )
    out = np.zeros((2, N, D), np.float64)
    for core in range(8):
        out[core // 4] += np.asarray(res.results[core]["y"], np.float64)
        out[core // 4, 1024:] += np.asarray(res.results[core]["y2"], np.float64)
    yf = out.astype(np.float32)
    if _trace:
        return yf, res
    return yf
